# revision 30
# baseline (speedup 1.0000x reference)
"""Trainium2 Bass kernel for nn_BatchRelationalModule.

Math (per batch element, see reference):
  featsT = [x_img[b].reshape(64, 256); arange(256)]            # [65, 256]
  pair MLP layer 0 decomposes: Wg0 @ concat(f_q, f_p) = u[:,q] + v[:,p]
    u = Wg0[:, :65] @ featsT + bg0, v = Wg0[:, 65:] @ featsT
  X0[p,q] = relu(u[:,q] + v[:,p])                              # 256x256 pairs
  X1 = relu(Wg1 @ X0 + bg1); X2 = relu(Wg2 @ X1 + bg2)
  S = sum_{p,q} X2;  out = Wo @ relu(Wp @ S + bp) + bo

Device layout: features (64) on partitions, pairs on free dim.  Two p-blocks
(p and p+128) are stacked to fill 128 partitions; block-diagonal [128,128]
fp16 weights process both halves in a SINGLE matmul per 512-col chunk (the
PE cost is free-size only, so fusing the halves halves tensor-engine time).
relu1/relu2 are split between DVE and ACT by column-count knobs; relu2 runs
in place on PSUM with accum_out producing the row sums.  The final Wp matmul
(K=128) folds the two halves.

Sharding: data-parallel over batch - 16 batches / 8 cores = 2 per core,
weights replicated, outputs gathered on host.
"""

from contextlib import ExitStack

import numpy as np

import concourse.bass as bass
import concourse.tile as tile
from concourse import bacc, mybir
from concourse.bass_utils import run_bass_kernel_spmd

F32 = mybir.dt.float32
F16 = mybir.dt.float16
N_CORES = 8
import os

FD_MAIN = 1024    # free dim of main tiles
# columns (per FD_MAIN tile) of relu1 / relu2 handled by DVE (rest on ACT)
R1_DVE = int(os.environ.get("R1_DVE", "448"))
R2_DVE = int(os.environ.get("R2_DVE", "320"))
X_BUFS = 4
B_PER_CORE = 2
L = 256  # h*w
C = 64
F = 64
D = C + 1  # 65

_CACHE = {}


def _build_nc(repeat=1):
    nc = bacc.Bacc(
        "TRN2",
        target_bir_lowering=False,
        debug=False,
        enable_asserts=False,
        num_devices=N_CORES,
    )

    # DRAM tensors (per-core inputs); xf is [D, B*L] (batches side by side,
    # coord row baked in host-side) so ONE DMA + ONE matmul pair covers the
    # whole per-body setup.  float32r tensors carry fp32 bits (np dtype
    # float32) but matmul at 1 cycle/row.
    F32R = mybir.dt.float32r
    xf = nc.dram_tensor("xf", [D, B_PER_CORE * L], F32R, kind="ExternalInput").ap()
    wg0lT_dd = nc.dram_tensor("wg0lT_dd", [D, 128], F32R, kind="ExternalInput").ap()
    wg0rT_dd = nc.dram_tensor("wg0rT_dd", [D, 128], F32R, kind="ExternalInput").ap()
    bg0dd = nc.dram_tensor("bg0dd", [128, 1], F32, kind="ExternalInput").ap()
    w1bd = nc.dram_tensor("w1bd", [128, 128], F16, kind="ExternalInput").ap()
    w2bd = nc.dram_tensor("w2bd", [128, 128], F16, kind="ExternalInput").ap()
    bg1dd = nc.dram_tensor("bg1dd", [128, 1], F32, kind="ExternalInput").ap()
    bg2dd = nc.dram_tensor("bg2dd", [128, 1], F32, kind="ExternalInput").ap()
    wpT_dd = nc.dram_tensor("wpT_dd", [128, F], F32, kind="ExternalInput").ap()
    bp_c = nc.dram_tensor("bp_c", [F, 1], F32, kind="ExternalInput").ap()
    woT = nc.dram_tensor("woT", [F, F], F32, kind="ExternalInput").ap()
    bo_c = nc.dram_tensor("bo_c", [F, 1], F32, kind="ExternalInput").ap()
    out = nc.dram_tensor("out", [B_PER_CORE, F, 1], F32, kind="ExternalOutput").ap()

    with tile.TileContext(nc) as tc, ExitStack() as ctx:
        consts = ctx.enter_context(tc.tile_pool(name="consts", bufs=1))
        setup = ctx.enter_context(tc.tile_pool(name="setup", bufs=2))
        xp = ctx.enter_context(tc.tile_pool(name="xp", bufs=X_BUFS))
        ps_bufs = 2
        ps1p = ctx.enter_context(
            tc.tile_pool(name="ps1p", bufs=ps_bufs, space="PSUM"))
        ps2p = ctx.enter_context(
            tc.tile_pool(name="ps2p", bufs=ps_bufs, space="PSUM"))
        accp = ctx.enter_context(tc.tile_pool(name="accp", bufs=2))
        pssp = ps1p  # setup-phase psum shares ps1 slots (tag below)

        def load_const(name, ap_in, shape, dt=F32):
            t = consts.tile(shape, dt, name=name)
            nc.sync.dma_start(t[:], ap_in)
            return t

        # fp32r: 1 cycle/row on PE (vs 4 for fp32) once free size >= 256
        F32R = mybir.dt.float32r
        wg0lT_sb = load_const("wg0lT_sb", wg0lT_dd, [D, 128], F32R)
        wg0rT_sb = load_const("wg0rT_sb", wg0rT_dd, [D, 128], F32R)
        bg0dd_sb = load_const("bg0dd_sb", bg0dd, [128, 1])
        w1bd_sb = load_const("w1bd_sb", w1bd, [128, 128], F16)
        w2bd_sb = load_const("w2bd_sb", w2bd, [128, 128], F16)
        bg1dd_sb = load_const("bg1dd_sb", bg1dd, [128, 1])
        bg2dd_sb = load_const("bg2dd_sb", bg2dd, [128, 1])
        wpT_dd_sb = load_const("wpT_dd_sb", wpT_dd, [128, F])
        bp_sb = load_const("bp_sb", bp_c, [F, 1])
        woT_sb = load_const("woT_sb", woT, [F, F])
        bo_sb = load_const("bo_sb", bo_c, [F, 1])

        zeros_sb = consts.tile([128, FD_MAIN], F16, name="zeros_sb")
        nc.vector.memset(zeros_sb[:], 0.0)

        # Warm the Relu activation table before the loop so the fixpoint
        # pass hoists LoadActFuncSet out of the body.
        warm = consts.tile([1, 1], F32, name="warm")
        nc.vector.memset(warm[:], 0.0)
        nc.scalar.activation(
            warm[:], warm[:], mybir.ActivationFunctionType.Relu
        )

        def body():
            _emit_body(
                nc, tc, setup, xp, ps1p, ps2p, pssp, accp,
                xf, out,
                wg0lT_sb, wg0rT_sb, bg0dd_sb, w1bd_sb, w2bd_sb,
                bg1dd_sb, bg2dd_sb, wpT_dd_sb, bp_sb, woT_sb, bo_sb,
                zeros_sb,
            )

        if repeat == 1:
            body()
        else:
            hint = (
                mybir.EngineType.PE,
                mybir.EngineType.DVE,
                mybir.EngineType.Activation,
                mybir.EngineType.SP,
                mybir.EngineType.Pool,
            )
            with tc.For_i(0, repeat, 1, hint_engines=hint, staggered_reset=True):
                body()

    nc.compile()
    return nc


def _emit_setup(nc, pssp, xf, wg0lT_sb, wg0rT_sb, bg0dd_sb,
                featsT, udup2, v2c):
    """Write the persistent setup tiles (featsT, udup2, v2c) for the next
    iteration: 1 DMA, 2 matmuls, 1 bias-add, 4 copies.  Emitted once as a
    prologue and again at the END of each body so the loop wraparound finds
    everything ready (cross-iteration prefetch; the repeat loop re-reads the
    same xf).  Bias-add on DVE, not ACT: ACT stays Relu-only so the
    activation table never reloads (Drain+LoadActFuncSet ~9us/body)."""
    add = mybir.AluOpType.add
    nc.sync.dma_start(featsT[:], xf)
    # u for both batches (duplicated on both partition halves by the M=128
    # stationary)
    ps_u = pssp.tile([128, B_PER_CORE * L], F32, name="ps_u", tag="ps1")
    nc.tensor.matmul(ps_u[:], wg0lT_sb[:], featsT[:], start=True, stop=True)
    nc.vector.tensor_scalar(udup2[:], ps_u[:], bg0dd_sb[:], None, op0=add)
    ps_v = pssp.tile([128, B_PER_CORE * L], F32, name="ps_v", tag="ps1")
    nc.tensor.matmul(ps_v[:], wg0rT_sb[:], featsT[:], start=True, stop=True)
    # v2c[:, b*128 + i] = [v_b[:, i] (top) ; v_b[:, 128+i] (bottom)]
    for b in range(B_PER_CORE):
        cs = slice(b * 128, (b + 1) * 128)
        nc.vector.tensor_copy(v2c[0:64, cs], ps_v[0:64, b * L : b * L + 128])
        nc.vector.tensor_copy(
            v2c[64:128, cs], ps_v[64:128, b * L + 128 : b * L + 256]
        )


def _emit_body(
    nc, tc, setup, xp, ps1p, ps2p, pssp, accp,
    xf, out,
    wg0lT_sb, wg0rT_sb, bg0dd_sb, w1bd_sb, w2bd_sb,
    bg1dd_sb, bg2dd_sb, wpT_dd_sb, bp_sb, woT_sb, bo_sb,
    zeros_sb, featsT, udup2, v2c,
):
    add = mybir.AluOpType.add
    mx = mybir.AluOpType.max
    Relu = mybir.ActivationFunctionType.Relu
    Ident = mybir.ActivationFunctionType.Identity

    FD = FD_MAIN       # free dim of the main tiles (FD/256 p-blocks per half)
    NB = FD // L       # p-blocks per half per iteration
    NITER = 128 // NB  # iterations per batch
    cd1 = R1_DVE
    cd2 = R2_DVE

    for b in range(B_PER_CORE):
        n_acc = NITER * ((1 if cd2 > 0 else 0) + (1 if cd2 < FD else 0))
        acc = accp.tile([128, n_acc], F32, name="acc", tag="acc")

        for i in range(NITER):
            # X0 = relu(u + v_p); block k covers p = 32k+i (top),
            # 128+32k+i (bottom)
            x0 = xp.tile([128, FD], F16, name="x0", tag="x0")
            for k in range(NB):
                vcol = b * 128 + NITER * k + i
                nc.vector.tensor_scalar(
                    x0[:, k * L : (k + 1) * L],
                    udup2[:, b * L : (b + 1) * L],
                    v2c[:, vcol : vcol + 1],
                    0.0, op0=add, op1=mx,
                )
            # layer 1: block-diagonal [128,128] fp16 stationary does both
            # halves in one matmul per 512-col chunk
            ps1 = ps1p.tile([128, FD], F32, name="ps1", tag="ps1")
            for c in range(FD // 512):
                cs = slice(512 * c, 512 * (c + 1))
                nc.tensor.matmul(
                    ps1[:, cs], w1bd_sb[:], x0[:, cs], start=True, stop=True,
                )
            # X1 = relu(ps1 + bg1): DVE takes cd1 columns, ACT the rest
            x1 = xp.tile([128, FD], F16, name="x1", tag="x1")
            if cd1 > 0:
                nc.vector.tensor_scalar(
                    x1[:, 0:cd1], ps1[:, 0:cd1], bg1dd_sb[:], 0.0,
                    op0=add, op1=mx,
                )
            if cd1 < FD:
                nc.scalar.activation(
                    x1[:, cd1:FD], ps1[:, cd1:FD], Relu, bias=bg1dd_sb[:]
                )
            # layer 2
            ps2 = ps2p.tile([128, FD], F32, name="ps2", tag="ps2")
            for c in range(FD // 512):
                cs = slice(512 * c, 512 * (c + 1))
                nc.tensor.matmul(
                    ps2[:, cs], w2bd_sb[:], x1[:, cs], start=True, stop=True,
                )
            # X2 = relu(ps2 + bg2) in place on PSUM; accum_out -> row sums.
            # DVE part must be scalar_tensor_tensor: with accum_out present,
            # tensor_scalar repurposes op1 as the REDUCTION op (row-max, and
            # no relu on out) — stt applies relu via op1 vs zeros and its
            # accum is a hardcoded add-reduce.
            if cd2 > 0:
                nc.vector.scalar_tensor_tensor(
                    ps2[:, 0:cd2], ps2[:, 0:cd2], bg2dd_sb[:],
                    zeros_sb[:, 0:cd2], op0=add, op1=mx,
                    accum_out=acc[:, 2 * i : 2 * i + 1] if cd2 < FD
                    else acc[:, i : i + 1],
                )
            if cd2 < FD:
                nc.scalar.activation(
                    ps2[:, cd2:FD], ps2[:, cd2:FD], Relu, bias=bg2dd_sb[:],
                    accum_out=acc[:, 2 * i + 1 : 2 * i + 2] if cd2 > 0
                    else acc[:, i : i + 1],
                )

        # Reduce accumulated columns -> [128, 1]
        accr = setup.tile([128, 1], F32, name="accr", tag="accr")
        nc.vector.tensor_reduce(
            accr[:], acc[:], axis=mybir.AxisListType.X, op=add
        )
        # f-network; K=128 matmul folds top+bottom halves of accr.
        # ps_h/ps_o live in the ps2 pool so the NEXT body's setup matmuls
        # can grab ps1 slots early; out-DMA goes via the idle Pool queue so
        # SP's queue holds only featsT loads (which then issue early).
        ps_h = ps2p.tile([F, 1], F32, name="ps_h", tag="ps2")
        nc.tensor.matmul(ps_h[:], wpT_dd_sb[:], accr[:], start=True, stop=True)
        h_sb = setup.tile([F, 1], F32, name="h_sb", tag="h_sb")
        nc.scalar.activation(h_sb[:], ps_h[:], Relu, bias=bp_sb[:])
        ps_o = ps2p.tile([F, 1], F32, name="ps_o", tag="ps2")
        nc.tensor.matmul(ps_o[:], woT_sb[:], h_sb[:], start=True, stop=True)
        o_sb = setup.tile([F, 1], F32, name="o_sb", tag="o_sb")
        nc.vector.tensor_scalar(o_sb[:], ps_o[:], bo_sb[:], None, op0=add)
        nc.scalar.dma_start(out[b], o_sb[:])


def _make_xf(x_img):
    """[bsz, C, h, w] image -> per-core [n_cores, D, B*L] fp32 with coord
    row baked in and the B_PER_CORE batches side by side in columns."""
    x = np.asarray(x_img, dtype=np.float32)
    bsz = x.shape[0]
    x = x.reshape(bsz, C, L)
    coord = np.broadcast_to(
        np.arange(L, dtype=np.float32).reshape(1, 1, L), (bsz, 1, L)
    )
    xd = np.concatenate([x, coord], axis=1)  # [bsz, D, L]
    # group per core: [n_cores, B, D, L] -> [n_cores, D, B*L]
    xc = xd.reshape(bsz // B_PER_CORE, B_PER_CORE, D, L)
    xc = xc.transpose(0, 2, 1, 3).reshape(bsz // B_PER_CORE, D, B_PER_CORE * L)
    return np.ascontiguousarray(xc)


def _block_diag_T(W):
    bd = np.zeros((128, 128), dtype=np.float16)
    wT = np.asarray(W, dtype=np.float16).T
    bd[0:64, 0:64] = wT
    bd[64:128, 64:128] = wT
    return bd


def _shared_in_map(Wg0, bg0, Wg1, bg1, Wg2, bg2, Wp, bp, Wo, bo):
    f = np.float32
    wg0l = np.ascontiguousarray(Wg0[:, :D].T, dtype=f)  # [65, 64]
    wg0r = np.ascontiguousarray(Wg0[:, D:].T, dtype=f)  # [65, 64]
    return {
        "wg0lT_dd": np.concatenate([wg0l, wg0l], axis=1),
        "wg0rT_dd": np.concatenate([wg0r, wg0r], axis=1),
        "bg0dd": np.concatenate([bg0, bg0]).astype(f).reshape(128, 1),
        "w1bd": _block_diag_T(Wg1),
        "w2bd": _block_diag_T(Wg2),
        "bg1dd": np.concatenate([bg1, bg1]).astype(f).reshape(128, 1),
        "bg2dd": np.concatenate([bg2, bg2]).astype(f).reshape(128, 1),
        "wpT_dd": np.concatenate([Wp.T, Wp.T], axis=0).astype(f),
        "bp_c": np.asarray(bp, f).reshape(F, 1),
        "woT": np.ascontiguousarray(Wo.T, dtype=f),
        "bo_c": np.asarray(bo, f).reshape(F, 1),
    }


def kernel(
    x_img, Wg0, bg0, Wg1, bg1, Wg2, bg2, Wp, bp, Wo, bo, trace=False, **run_kwargs
):
    if "nc" not in _CACHE:
        _CACHE["nc"] = _build_nc()
    nc = _CACHE["nc"]

    shared = _shared_in_map(
        np.asarray(Wg0), np.asarray(bg0), np.asarray(Wg1), np.asarray(bg1),
        np.asarray(Wg2), np.asarray(bg2), np.asarray(Wp), np.asarray(bp),
        np.asarray(Wo), np.asarray(bo),
    )
    x = _make_xf(x_img)

    in_maps = []
    for core in range(N_CORES):
        m = dict(shared)
        m["xf"] = x[core]
        in_maps.append(m)

    res = run_bass_kernel_spmd(
        nc, in_maps, core_ids=list(range(N_CORES)), trace=trace, **run_kwargs
    )
    outs = [r["out"].reshape(B_PER_CORE, F) for r in res.results]
    full = np.concatenate(outs, axis=0)
    if trace:
        _CACHE["last_results"] = res
    return full
